# revision 13
# baseline (speedup 1.0000x reference)
"""Trainium2 Bass kernel for nn_GCLSTMModel_48868137894020.

The reference runs ONE GCLSTM cell step per layer with H0 = C0 = 0, so the
cheb weights / Laplacian / forget gate / peep[0:2] are algebraically dead.
What remains per layer (d = 140 then 280), with X the layer input:
  I = sigmoid(X @ W[0] + cb[0] + b[0])
  T = tanh   (X @ W[2] + cb[2] + b[2])
  C = I * T
  O = sigmoid(X @ W[3] + cb[3] + b[3] + peep[2] * C)
  X' = relu(O * tanh(C)) = O * max(tanh(C), 0)
then out = relu(X'' @ fc_w + fc_b).

v2 strategy (vs the f32r baseline at ~28.2us):
  - tanh-ONLY gates -> a single ACT table load (sigmoid+tanh needed two):
      sigma(z) = (tanh(z/2)+1)/2, so with u_a=tanh(a/2), u_b=tanh(b):
      D := (u_a+1)*u_b = 2C;  po := c0 + (pe/2)*D;  u_c=tanh(po/2);
      u_d=tanh(D/2)=tanh(C);  x' := (u_c+1)*max(u_d,0) = 2x.
    The factor 2 on x is folded into the next layer's weights (W2/2, fc_w/2)
    on the host (parameter-only transform).
  - bf16 everywhere on chip (PSUM stays f32): halves matmul passes,
    LDWEIGHTS time, DVE element cost and DMA bytes. numpy-checked rel err
    ~3e-3 (gate is 2e-2).
  - NO DMAs issued from the Scalar engine (they serialized with the ACT
    table loads in the baseline); inputs ride 2 packed HWDGE DMAs on Sync
    plus one SWDGE broadcast on GpSimd.
  - L1 computed TRANSPOSED with chunk-MERGED gate math on [128, 72] tiles
    (chunk c0 = features 0:128 at cols 0:36, c1 = 128:140 at cols 36:72);
    per-feature biases are folded into the matmul via a ones-row appended
    to adj (row 35), so merged ACT ops need no per-partition bias.
  - L2 computed node-major [35, 280] with merged full-width gates; biases
    fold in via ones-row 12 of the small x1 chunk.
  - PE warm-up matmuls run during the initial DMA window so the HAM clock
    gate reaches 2.4 GHz before the L2/transpose/FC matmuls.

Sharding: the problem is tiny (N=35); all 8 cores run the identical
program on replicated inputs (no collectives), output taken from core 0.
"""

import sys

for _p in ("/opt/trn_rl_repo", "/opt/pypackages"):
    if _p not in sys.path:
        sys.path.append(_p)

from contextlib import ExitStack

import numpy as np

import concourse.bacc as bacc
import concourse.bass as bass
import concourse.mybir as mybir
import concourse.tile as tile

F32 = mybir.dt.float32
BF16 = mybir.dt.bfloat16
AF = mybir.ActivationFunctionType
OP = mybir.AluOpType
GATES = (0, 2, 3)  # I, T (cell), O - forget gate (1) is dead
N = 35
D1 = 140
D2 = 280
N_CORES = 8

# w1pack free-dim layout (36 partitions):
#   [0:36)      adj_ext: rows 0:35 = adj, row 35 = ones (bias row), col 35 = 0
#   [36:72)     identity 35x35 (for PE transposes)
#   [72:492)    W1ext gates (I,T,O): rows 0:35 = W1[g], row 35 = cb1[g]+b1[g]
#   [492:1332)  W2b-stack: per gate [14, 280]: rows 0:12 = W2[g][128:140]/2,
#               row 12 = cb2[g]+b2[g], row 13 = 0   (rows 14:36 zero)
#   [1332:1368) x1b region: rows 0:12 overwritten on-chip with x1' chunk1,
#               row 12 = ones (host), row 13 = 0 (host)
W1COLS = 1368
# w2pack free-dim layout (128 partitions):
#   [0:840)     W2a gates (I,T,O): W2[g][0:128, :]/2
#   [840:842)   pe1 cols: col 840 = pe1[2][0:128]/2, col 841 = pe1[2][128:140]/2
#   [842:914)   fcw chunks: [128, 36] x2 = fc_w[0:128]/2, fc_w[128:256]/2
#   [914:950)   fcx [25, 36]: rows 0:24 = fc_w[256:280]/2, row 24 = fc_b
#   [950:1230)  pe2 row: pe2[2]/2 (broadcast-DMA'd to [35, 280])
#   [1230:1266) x2T2 region: rows 0:24 overwritten on-chip with x2T chunk 2,
#               row 24 = ones (host)
W2COLS = 1266

WARM_PRE = 5  # N=512 warm matmuls before L1 (fill the input-DMA window)
WARM_MID = 3  # more between L1 and L2 matmuls to cross the HAM window


def build_nc() -> bass.Bass:
    nc = bacc.Bacc()

    w1p = nc.dram_tensor("w1p", [36, W1COLS], BF16, kind="ExternalInput")
    w2p = nc.dram_tensor("w2p", [128, W2COLS], BF16, kind="ExternalInput")
    out = nc.dram_tensor("out", [N, N], F32, kind="ExternalOutput")

    with ExitStack() as ctx:
        tc = ctx.enter_context(tile.TileContext(nc))
        sb = ctx.enter_context(tc.tile_pool(name="sb", bufs=1))
        psA = ctx.enter_context(tc.tile_pool(name="psA", bufs=5, space="PSUM"))
        psB = ctx.enter_context(tc.tile_pool(name="psB", bufs=3, space="PSUM"))

        # ---- input DMAs: both packed loads on the Sync HWDGE ring ----
        w1sb = sb.tile([36, W1COLS], BF16, tag="w1sb")
        nc.sync.dma_start(out=w1sb, in_=w1p[:, :])
        w2sb = sb.tile([128, W2COLS], BF16, tag="w2sb")
        nc.sync.dma_start(out=w2sb, in_=w2p[:, :])
        pe2t = sb.tile([N, D2], BF16, tag="pe2t")
        nc.gpsimd.dma_start(
            out=pe2t, in_=w2p[0:1, 950:1230].to_broadcast([N, D2])
        )

        adj_v = w1sb[:, 0:36]           # [36, 36] rhs for L1 (row 35 = ones)
        ident_v = w1sb[0:35, 36:71]     # [35, 35] identity for transposes

        # ---- constants ----
        warm_w = sb.tile([1, 512], BF16, tag="warm_w")
        nc.vector.memset(warm_w[:, :], 1.0)
        # x1b / x2T2 live inside the packed weight tiles: their ones/zero
        # rows are host-packed, the data rows are overwritten on-chip.

        # ---- PE warm-up: release the HAM clock gate during the DMA wait ----
        ps_warm = psA.tile([1, 512], F32, tag="psA", name="ps_warm")
        warm_pre_last = None
        for i in range(WARM_PRE):
            warm_pre_last = nc.tensor.matmul(
                ps_warm, lhsT=warm_w[0:1, 0:1], rhs=warm_w[0:1, :],
                start=True, stop=True,
            )

        # ---- layer 1, transposed + chunk-merged: x1T[f, n] ----
        # psum [128, 72]: block c0 = features 0:128 at cols 0:36,
        #                 block c1 = features 128:140 at cols 36:72 (rows 0:12)
        chunks1 = ((0, 0, 128), (1, 128, 140))
        ps1 = {}
        mm1_first = None
        mm1_last = None
        for k, g in enumerate(GATES):
            ps1[k] = psA.tile([128, 72], F32, tag="psA", name=f"ps1_{k}")
            for ci, a, b in chunks1:
                mm1_last = nc.tensor.matmul(
                    ps1[k][0 : b - a, 36 * ci : 36 * ci + 36],
                    lhsT=w1sb[:, 72 + k * D1 + a : 72 + k * D1 + b],
                    rhs=adj_v,
                    start=True,
                    stop=True,
                )
                if mm1_first is None:
                    mm1_first = mm1_last
        # keep warm-up ahead of the real matmuls in the PE stream
        tile.add_dep_helper(
            mm1_first.ins, warm_pre_last.ins, sync=False, reason="warm first"
        )

        ua1 = sb.tile([128, 72], BF16, tag="ua1")
        nc.scalar.activation(ua1, ps1[0][:, :], AF.Tanh, scale=0.5)
        ub1 = sb.tile([128, 72], BF16, tag="ub1")
        nc.scalar.activation(ub1, ps1[1][:, :], AF.Tanh)
        d1 = sb.tile([128, 72], BF16, tag="d1")
        nc.vector.scalar_tensor_tensor(
            d1, in0=ua1, scalar=1.0, in1=ub1, op0=OP.add, op1=OP.mult
        )
        ud1 = sb.tile([128, 72], BF16, tag="ud1")
        nc.scalar.activation(ud1, d1, AF.Tanh, scale=0.5)
        # po = psO + (pe1/2) * D, per chunk (pe is a per-partition scalar col)
        po1 = sb.tile([128, 72], BF16, tag="po1")
        nc.vector.scalar_tensor_tensor(
            po1[:, 0:36], in0=d1[:, 0:36], scalar=w2sb[:, 840:841],
            in1=ps1[2][:, 0:36], op0=OP.mult, op1=OP.add,
        )
        nc.vector.scalar_tensor_tensor(
            po1[0:12, 36:72], in0=d1[0:12, 36:72], scalar=w2sb[0:12, 841:842],
            in1=ps1[2][0:12, 36:72], op0=OP.mult, op1=OP.add,
        )
        uc1 = sb.tile([128, 72], BF16, tag="uc1")
        nc.scalar.activation(uc1, po1, AF.Tanh, scale=0.5)
        t1 = sb.tile([128, 72], BF16, tag="t1")
        nc.vector.tensor_scalar_max(t1, ud1, 0.0)
        # x' = (u_c + 1) * t  (= 2x; the /2 is folded into W2 on the host)
        x1a = sb.tile([128, 36], BF16, tag="x1a")
        nc.vector.scalar_tensor_tensor(
            x1a, in0=uc1[:, 0:36], scalar=1.0, in1=t1[:, 0:36],
            op0=OP.add, op1=OP.mult,
        )
        nc.vector.scalar_tensor_tensor(
            w1sb[0:12, 1332:1368], in0=uc1[0:12, 36:72], scalar=1.0,
            in1=t1[0:12, 36:72], op0=OP.add, op1=OP.mult,
        )

        # ---- mid warm-up (PE idles through the L1 gate chain) ----
        warm_mid_last = None
        for i in range(WARM_MID):
            warm_mid_last = nc.tensor.matmul(
                ps_warm, lhsT=warm_w[0:1, 0:1], rhs=warm_w[0:1, :],
                start=True, stop=True,
            )
            tile.add_dep_helper(
                warm_mid_last.ins, mm1_last.ins, sync=False, reason="after L1"
            )

        # ---- layer 2, node-major: x2[n, f2] = x1'.T @ W2' ----
        ps2 = {}
        mm2_first = None
        for k, g in enumerate(GATES):
            ps2[k] = psB.tile([N, D2], F32, tag="psB", name=f"ps2_{k}")
            mm = nc.tensor.matmul(
                ps2[k], lhsT=x1a[:, 0:35], rhs=w2sb[:, k * D2 : (k + 1) * D2],
                start=True, stop=False,
            )
            if mm2_first is None:
                mm2_first = mm
            nc.tensor.matmul(
                ps2[k], lhsT=w1sb[0:14, 1332:1367],
                rhs=w1sb[0:14, 492 + k * D2 : 492 + (k + 1) * D2],
                start=False, stop=True,
            )
        tile.add_dep_helper(
            mm2_first.ins, warm_mid_last.ins, sync=False, reason="warm first"
        )

        ua2 = sb.tile([N, D2], BF16, tag="ua2")
        nc.scalar.activation(ua2, ps2[0], AF.Tanh, scale=0.5)
        ub2 = sb.tile([N, D2], BF16, tag="ub2")
        nc.scalar.activation(ub2, ps2[1], AF.Tanh)
        d2 = sb.tile([N, D2], BF16, tag="d2")
        nc.vector.scalar_tensor_tensor(
            d2, in0=ua2, scalar=1.0, in1=ub2, op0=OP.add, op1=OP.mult
        )
        ud2 = sb.tile([N, D2], BF16, tag="ud2")
        nc.scalar.activation(ud2, d2, AF.Tanh, scale=0.5)
        pc2 = sb.tile([N, D2], BF16, tag="pc2")
        nc.vector.tensor_mul(pc2, d2, pe2t)
        po2 = sb.tile([N, D2], BF16, tag="po2")
        nc.vector.tensor_add(po2, pc2, ps2[2])
        uc2 = sb.tile([N, D2], BF16, tag="uc2")
        nc.scalar.activation(uc2, po2, AF.Tanh, scale=0.5)
        t2 = sb.tile([N, D2], BF16, tag="t2")
        nc.vector.tensor_scalar_max(t2, ud2, 0.0)
        x2 = sb.tile([N, D2], BF16, tag="x2")
        x2_stt = nc.vector.scalar_tensor_tensor(
            x2, in0=uc2, scalar=1.0, in1=t2, op0=OP.add, op1=OP.mult
        )

        # ---- transpose x2 (PE, bf16 single-pass), then FC ----
        psTs = []
        t_last = None
        for j, (a, b) in enumerate(((0, 128), (128, 256), (256, 280))):
            psT = psA.tile([b - a, N], BF16, tag="psA", name=f"psT{j}")
            t_last = nc.tensor.transpose(psT, x2[:, a:b], ident_v)
            psTs.append(psT)
        x2T0 = sb.tile([128, N], BF16, tag="x2T0")
        x2T1 = sb.tile([128, N], BF16, tag="x2T1")
        x2T2_dst = w2sb[0:24, 1230:1265]
        for j, dst in enumerate((x2T0[:, :], x2T1[:, :], x2T2_dst)):
            cp = nc.vector.tensor_copy(dst, psTs[j])
            tile.add_dep_helper(
                cp.ins, x2_stt.ins, sync=False, reason="casts after x2"
            )
        psfc = psB.tile([N, 36], F32, tag="psB", name="psfc")
        fc1 = nc.tensor.matmul(
            psfc, lhsT=x2T0[:, :], rhs=w2sb[:, 842:878], start=True, stop=False
        )
        tile.add_dep_helper(
            fc1.ins, t_last.ins, sync=False, reason="transposes before FC"
        )
        nc.tensor.matmul(
            psfc, lhsT=x2T1[:, :], rhs=w2sb[:, 878:914], start=False, stop=False
        )
        nc.tensor.matmul(
            psfc, lhsT=w2sb[0:25, 1230:1265], rhs=w2sb[0:25, 914:950],
            start=False, stop=True,
        )
        out_sb = sb.tile([N, N], F32, tag="out_sb")
        nc.vector.tensor_scalar_max(out_sb, psfc[:, 0:N], 0.0)
        nc.sync.dma_start(out=out[:, :], in_=out_sb)

    nc.compile()
    return nc


def pack_inputs(
    adj_matrix, W1, cheb1_b, peep1, b1, W2, cheb2_b, peep2, b2, fc_w, fc_b
) -> dict:
    """Host-side packing: gather/concat + parameter-only transforms
    (bias sums, the x'=2x scale fold into W2/fc_w, bf16 cast)."""
    import ml_dtypes

    bf = ml_dtypes.bfloat16
    f = np.float32

    w1p_h = np.zeros((36, W1COLS), dtype=f)
    w1p_h[0:35, 0:35] = adj_matrix
    w1p_h[35, 0:35] = 1.0
    w1p_h[0:35, 36:71] = np.eye(N, dtype=f)
    for k, g in enumerate(GATES):
        w1p_h[0:35, 72 + k * D1 : 72 + (k + 1) * D1] = W1[g]
        w1p_h[35, 72 + k * D1 : 72 + (k + 1) * D1] = cheb1_b[g] + b1[g]
    for k, g in enumerate(GATES):
        c = 492 + k * D2
        w1p_h[0:12, c : c + D2] = W2[g][128:140] * 0.5
        w1p_h[12, c : c + D2] = cheb2_b[g] + b2[g]
    w1p_h[12, 1332:1368] = 1.0  # x1b ones row (adds the bias2 rows)

    w2p_h = np.zeros((128, W2COLS), dtype=f)
    for k, g in enumerate(GATES):
        w2p_h[:, k * D2 : (k + 1) * D2] = W2[g][0:128] * 0.5
    w2p_h[:, 840] = peep1[2][0:128] * 0.5
    w2p_h[0:12, 841] = peep1[2][128:140] * 0.5
    w2p_h[:, 842:877] = fc_w[0:128] * 0.5
    w2p_h[:, 878:913] = fc_w[128:256] * 0.5
    w2p_h[0:24, 914:949] = fc_w[256:280] * 0.5
    w2p_h[24, 914:949] = fc_b
    w2p_h[0, 950:1230] = peep2[2] * 0.5
    w2p_h[24, 1230:1265] = 1.0  # x2T2 ones row (adds fc_b)

    return {
        "w1p": np.ascontiguousarray(w1p_h.astype(bf)),
        "w2p": np.ascontiguousarray(w2p_h.astype(bf)),
    }


_NC_CACHE: list = []


def kernel(
    adj_matrix,
    W1,
    cheb1_W,
    cheb1_b,
    peep1,
    b1,
    W2,
    cheb2_W,
    cheb2_b,
    peep2,
    b2,
    fc_w,
    fc_b,
) -> np.ndarray:
    from concourse.bass_utils import run_bass_kernel_spmd

    in_map = pack_inputs(
        adj_matrix, W1, cheb1_b, peep1, b1, W2, cheb2_b, peep2, b2, fc_w, fc_b
    )

    if not _NC_CACHE:
        _NC_CACHE.append(build_nc())
    nc = _NC_CACHE[0]

    in_maps = [dict(in_map) for _ in range(N_CORES)]
    res = run_bass_kernel_spmd(nc, in_maps, core_ids=list(range(N_CORES)))
    return np.asarray(res.results[0]["out"], dtype=np.float32)


# revision 14
# speedup vs baseline: 1.0420x; 1.0420x over previous
"""Trainium2 Bass kernel for nn_GCLSTMModel_48868137894020.

The reference runs ONE GCLSTM cell step per layer with H0 = C0 = 0, so the
cheb weights / Laplacian / forget gate / peep[0:2] are algebraically dead.
What remains per layer (d = 140 then 280), with X the layer input:
  I = sigmoid(X @ W[0] + cb[0] + b[0])
  T = tanh   (X @ W[2] + cb[2] + b[2])
  C = I * T
  O = sigmoid(X @ W[3] + cb[3] + b[3] + peep[2] * C)
  X' = relu(O * tanh(C)) = O * max(tanh(C), 0)
then out = relu(X'' @ fc_w + fc_b).

v3 design (baseline f32r kernel measured 28.2us; ~15us of that is fixed
NEFF preamble + semaphore-teardown tail, so the win is all in the body):

  - tanh-ONLY activations -> a single ACT table load (sigmoid+tanh cost
    two 1.28us loads): sigma(z) = (tanh(z/2)+1)/2. With u_a = tanh(a/2),
    u_b = tanh(b):  D := (u_a+1)*u_b = 2C,  po := c0 + (pe/2)*D,
    u_c = tanh(po/2),  tp = tanh(max(D,0)/2) = max(tanh(C), 0)  (tanh is
    monotone, tanh(0)=0),  x' := (u_c+1)*tp = 2x.  The factor 2 on x is
    folded into the next layer's weights (W2/2, fc_w/2) on the host.
  - per layer only TWO activation instructions: the I and T pre-acts
    share one PSUM tile (I's 0.5 input scale is folded into its weights)
    so one tanh covers both; dp=max(D,0) and po share one SBUF tile so a
    second tanh(0.5*) covers u_c and tp together.  ACT's ~293ns fixed
    cost per instruction dominates these small ops, so merging wins.
  - bf16 everywhere on chip (PSUM f32): single-pass matmuls, half DMA.
    numpy-checked end-to-end rel err ~3e-3 (gate is 2e-2).
  - BOTH layers computed transposed (features on partitions): per-feature
    biases fold into the matmuls via ones-rows of the moving operand, the
    peephole is a plain tensor op, gate DVE/ACT ops run 128 lanes wide,
    and the FC consumes x2T directly -- no transposes at all.
  - no DMAs on the Scalar engine except a tiny pe1 load after the table
    load (they serialized with table loads in the baseline).

Sharding: the problem is tiny (N=35); all 8 cores run the identical
program on replicated inputs (no collectives), output taken from core 0.
"""

import sys

for _p in ("/opt/trn_rl_repo", "/opt/pypackages"):
    if _p not in sys.path:
        sys.path.append(_p)

from contextlib import ExitStack

import numpy as np

import concourse.bacc as bacc
import concourse.bass as bass
import concourse.mybir as mybir
import concourse.tile as tile

F32 = mybir.dt.float32
BF16 = mybir.dt.bfloat16
AF = mybir.ActivationFunctionType
OP = mybir.AluOpType
GATES = (0, 2, 3)  # I, T (cell), O - forget gate (1) is dead
N = 35
D1 = 140
D2 = 280
N_CORES = 8

# w1e (36 partitions): [0:36) adj_ext (row 35 = ones, col 35 = 0);
#   [36:456) W1ext gates I,T,O: rows 0:35 = W1[g] (I halved), row 35 =
#   cb1[g]+b1[g] (I's halved).
W1ECOLS = 456
# w1b (14 partitions): [0:840) W2b-stack per gate [14, 280]: rows 0:12 =
#   W2[g][128:140] * s_g (I: 1/4, T/O: 1/2), row 12 = cb2[g]+b2[g] (I's
#   halved), row 13 = 0;  [840:876) x1b region: rows 0:12 overwritten
#   on-chip with x1' chunk1, row 12 = ones, row 13 = 0.
W1BCOLS = 876
# w2p (128 partitions): [0:840) W2a gates I,T,O = W2[g][0:128] * s_g;
#   [840:912) fcw chunks [128, 36] x2 = fc_w[0:128]/2, fc_w[128:256]/2;
#   [912:948) fcx [25, 36]: rows 0:24 = fc_w[256:280]/2, row 24 = fc_b;
#   [948:1056) pe2t [128, 108]: block c col j: pe2[2][128c + p]/2;
#   [1056:1164) x2t region: zeros, except row 24 cols 1128:1163 = ones.
W2COLS = 1164
X2T0 = 1056  # x2t region base col


def build_nc() -> bass.Bass:
    nc = bacc.Bacc()

    w1e = nc.dram_tensor("w1e", [36, W1ECOLS], BF16, kind="ExternalInput")
    w1b = nc.dram_tensor("w1b", [14, W1BCOLS], BF16, kind="ExternalInput")
    w2p = nc.dram_tensor("w2p", [128, W2COLS], BF16, kind="ExternalInput")
    pe1 = nc.dram_tensor("pe1", [128, 2], BF16, kind="ExternalInput")
    out = nc.dram_tensor("out", [N, N], F32, kind="ExternalOutput")

    with ExitStack() as ctx:
        tc = ctx.enter_context(tile.TileContext(nc))
        sb = ctx.enter_context(tc.tile_pool(name="sb", bufs=1))
        psA = ctx.enter_context(tc.tile_pool(name="psA", bufs=2, space="PSUM"))
        psB = ctx.enter_context(tc.tile_pool(name="psB", bufs=3, space="PSUM"))

        # ---- input DMAs ----
        w1esb = sb.tile([36, W1ECOLS], BF16, tag="w1esb")
        nc.sync.dma_start(out=w1esb, in_=w1e[:, :])
        w2sb = sb.tile([128, W2COLS], BF16, tag="w2sb")
        nc.sync.dma_start(out=w2sb, in_=w2p[:, :])
        w1bsb = sb.tile([14, W1BCOLS], BF16, tag="w1bsb")
        nc.gpsimd.dma_start(out=w1bsb, in_=w1b[:, :])
        pe1sb = sb.tile([128, 2], BF16, tag="pe1sb")
        nc.scalar.dma_start(out=pe1sb, in_=pe1[:, :])

        adj_v = w1esb[:, 0:36]  # [36, 36] rhs for L1 (row 35 = ones)

        # ---- layer 1, transposed, chunk-merged ----
        # psIT1 [128, 144]: I blocks at cols 0:72, T at 72:144
        #   (block c0 = features 0:128, c1 = 128:140 on rows 0:12)
        psIT1 = psA.tile([128, 144], F32, tag="psA", name="psIT1")
        psO1 = psA.tile([128, 72], F32, tag="psA", name="psO1")
        chunks1 = ((0, 0, 128), (1, 128, 140))
        for k in range(3):  # I, T, O
            for ci, a, b in chunks1:
                dst = (
                    psIT1[0 : b - a, 72 * k + 36 * ci : 72 * k + 36 * ci + 36]
                    if k < 2
                    else psO1[0 : b - a, 36 * ci : 36 * ci + 36]
                )
                nc.tensor.matmul(
                    dst,
                    lhsT=w1esb[:, 36 + k * D1 + a : 36 + k * D1 + b],
                    rhs=adj_v,
                    start=True,
                    stop=True,
                )

        uab1 = sb.tile([128, 144], BF16, tag="uab1")
        nc.scalar.activation(uab1, psIT1[:, :], AF.Tanh)
        d1 = sb.tile([128, 72], BF16, tag="d1")
        nc.vector.scalar_tensor_tensor(
            d1, in0=uab1[:, 0:72], scalar=1.0, in1=uab1[:, 72:144],
            op0=OP.add, op1=OP.mult,
        )
        # dpo1: dp = max(D,0) at cols 0:72, po = c0 + (pe1/2)*D at 72:144
        dpo1 = sb.tile([128, 144], BF16, tag="dpo1")
        nc.vector.tensor_scalar_max(dpo1[:, 0:72], d1, 0.0)
        nc.vector.scalar_tensor_tensor(
            dpo1[:, 72:108], in0=d1[:, 0:36], scalar=pe1sb[:, 0:1],
            in1=psO1[:, 0:36], op0=OP.mult, op1=OP.add,
        )
        nc.vector.scalar_tensor_tensor(
            dpo1[0:12, 108:144], in0=d1[0:12, 36:72], scalar=pe1sb[0:12, 1:2],
            in1=psO1[0:12, 36:72], op0=OP.mult, op1=OP.add,
        )
        tcp1 = sb.tile([128, 144], BF16, tag="tcp1")
        nc.scalar.activation(tcp1, dpo1, AF.Tanh, scale=0.5)
        # x' = (u_c + 1) * tp   (tp = tcp1 cols 0:72, u_c = cols 72:144)
        x1a = sb.tile([128, 36], BF16, tag="x1a")
        nc.vector.scalar_tensor_tensor(
            x1a, in0=tcp1[:, 72:108], scalar=1.0, in1=tcp1[:, 0:36],
            op0=OP.add, op1=OP.mult,
        )
        nc.vector.scalar_tensor_tensor(
            w1bsb[0:12, 840:876], in0=tcp1[0:12, 108:144], scalar=1.0,
            in1=tcp1[0:12, 36:72], op0=OP.add, op1=OP.mult,
        )
        x1b_v = w1bsb[0:14, 840:876]

        # ---- layer 2, transposed: x2T[f2, n] ----
        # psIT2 [128, 216]: I chunk-blocks at cols 0:108, T at 108:216;
        # chunk c0 = f2 0:128, c1 = 128:256, c2 = 256:280 (rows 0:24).
        psIT2 = psB.tile([128, 216], F32, tag="psB", name="psIT2")
        psO2 = psB.tile([128, 108], F32, tag="psB", name="psO2")
        chunks2 = ((0, 0, 128), (1, 128, 256), (2, 256, 280))
        for k in range(3):  # I, T, O
            for ci, a, b in chunks2:
                if k < 2:
                    dst = psIT2[0 : b - a, 108 * k + 36 * ci : 108 * k + 36 * ci + 36]
                else:
                    dst = psO2[0 : b - a, 36 * ci : 36 * ci + 36]
                nc.tensor.matmul(
                    dst,
                    lhsT=w2sb[:, k * D2 + a : k * D2 + b],
                    rhs=x1a[:, :],
                    start=True,
                    stop=False,
                )
                nc.tensor.matmul(
                    dst,
                    lhsT=w1bsb[0:14, k * D2 + a : k * D2 + b],
                    rhs=x1b_v,
                    start=False,
                    stop=True,
                )

        uab2 = sb.tile([128, 216], BF16, tag="uab2")
        nc.scalar.activation(uab2, psIT2[:, :], AF.Tanh)
        d2 = sb.tile([128, 108], BF16, tag="d2")
        nc.vector.scalar_tensor_tensor(
            d2, in0=uab2[:, 0:108], scalar=1.0, in1=uab2[:, 108:216],
            op0=OP.add, op1=OP.mult,
        )
        dpo2 = sb.tile([128, 216], BF16, tag="dpo2")
        nc.vector.tensor_scalar_max(dpo2[:, 0:108], d2, 0.0)
        pc2 = sb.tile([128, 108], BF16, tag="pc2")
        nc.vector.tensor_mul(pc2, d2, w2sb[:, 948:1056])
        nc.vector.tensor_add(dpo2[:, 108:216], pc2, psO2)
        tcp2 = sb.tile([128, 216], BF16, tag="tcp2")
        nc.scalar.activation(tcp2, dpo2, AF.Tanh, scale=0.5)
        # x2' = (u_c + 1) * tp into the w2sb x2t region (ones row at
        # partition 24 of block 2 is host-packed and left unwritten)
        x2a = nc.vector.scalar_tensor_tensor(
            w2sb[:, X2T0 : X2T0 + 72], in0=tcp2[:, 108:180], scalar=1.0,
            in1=tcp2[:, 0:72], op0=OP.add, op1=OP.mult,
        )
        nc.vector.scalar_tensor_tensor(
            w2sb[0:24, X2T0 + 72 : X2T0 + 108], in0=tcp2[0:24, 180:216],
            scalar=1.0, in1=tcp2[0:24, 72:108], op0=OP.add, op1=OP.mult,
        )

        # ---- FC: psfc += x2T_c.T @ fcw_c (bias via ones row of chunk 2) ----
        psfc = psB.tile([N, 36], F32, tag="psB", name="psfc")
        nc.tensor.matmul(
            psfc, lhsT=w2sb[0:128, X2T0 : X2T0 + 35], rhs=w2sb[:, 840:876],
            start=True, stop=False,
        )
        nc.tensor.matmul(
            psfc, lhsT=w2sb[0:128, X2T0 + 36 : X2T0 + 71], rhs=w2sb[:, 876:912],
            start=False, stop=False,
        )
        nc.tensor.matmul(
            psfc, lhsT=w2sb[0:25, X2T0 + 72 : X2T0 + 107],
            rhs=w2sb[0:25, 912:948], start=False, stop=True,
        )
        out_sb = sb.tile([N, N], F32, tag="out_sb")
        nc.vector.tensor_scalar_max(out_sb, psfc[:, 0:N], 0.0)
        nc.sync.dma_start(out=out[:, :], in_=out_sb)

    nc.compile()
    return nc


def pack_inputs(
    adj_matrix, W1, cheb1_b, peep1, b1, W2, cheb2_b, peep2, b2, fc_w, fc_b
) -> dict:
    """Host-side packing: gather/concat + parameter-only transforms
    (bias sums, sigma->tanh 0.5 folds, the x'=2x fold, bf16 cast)."""
    import ml_dtypes

    bf = ml_dtypes.bfloat16
    f = np.float32
    # per-gate input scale: I folds sigma's 0.5; all of L2 folds x'=2x
    s1 = {0: 0.5, 2: 1.0, 3: 1.0}
    s2 = {0: 0.25, 2: 0.5, 3: 0.5}
    sb_ = {0: 0.5, 2: 1.0, 3: 1.0}  # L2 bias scale (I halved)

    w1e_h = np.zeros((36, W1ECOLS), dtype=f)
    w1e_h[0:35, 0:35] = adj_matrix
    w1e_h[35, 0:35] = 1.0
    for k, g in enumerate(GATES):
        c = 36 + k * D1
        w1e_h[0:35, c : c + D1] = W1[g] * s1[g]
        w1e_h[35, c : c + D1] = (cheb1_b[g] + b1[g]) * s1[g]

    w1b_h = np.zeros((14, W1BCOLS), dtype=f)
    for k, g in enumerate(GATES):
        c = k * D2
        w1b_h[0:12, c : c + D2] = W2[g][128:140] * s2[g]
        w1b_h[12, c : c + D2] = (cheb2_b[g] + b2[g]) * sb_[g]
    w1b_h[12, 840:876] = 1.0

    w2p_h = np.zeros((128, W2COLS), dtype=f)
    for k, g in enumerate(GATES):
        w2p_h[:, k * D2 : (k + 1) * D2] = W2[g][0:128] * s2[g]
    w2p_h[:, 840:875] = fc_w[0:128] * 0.5
    w2p_h[:, 876:911] = fc_w[128:256] * 0.5
    w2p_h[0:24, 912:947] = fc_w[256:280] * 0.5
    w2p_h[24, 912:947] = fc_b
    for c in range(3):
        n_f = 128 if c < 2 else 24
        w2p_h[0:n_f, 948 + 36 * c : 948 + 36 * (c + 1)] = (
            peep2[2][128 * c : 128 * c + n_f, None] * 0.5
        )
    w2p_h[24, X2T0 + 72 : X2T0 + 107] = 1.0  # x2t chunk-2 ones row

    pe1_h = np.zeros((128, 2), dtype=f)
    pe1_h[:, 0] = peep1[2][0:128] * 0.5
    pe1_h[0:12, 1] = peep1[2][128:140] * 0.5

    return {
        "w1e": np.ascontiguousarray(w1e_h.astype(bf)),
        "w1b": np.ascontiguousarray(w1b_h.astype(bf)),
        "w2p": np.ascontiguousarray(w2p_h.astype(bf)),
        "pe1": np.ascontiguousarray(pe1_h.astype(bf)),
    }


_NC_CACHE: list = []


def kernel(
    adj_matrix,
    W1,
    cheb1_W,
    cheb1_b,
    peep1,
    b1,
    W2,
    cheb2_W,
    cheb2_b,
    peep2,
    b2,
    fc_w,
    fc_b,
) -> np.ndarray:
    from concourse.bass_utils import run_bass_kernel_spmd

    in_map = pack_inputs(
        adj_matrix, W1, cheb1_b, peep1, b1, W2, cheb2_b, peep2, b2, fc_w, fc_b
    )

    if not _NC_CACHE:
        _NC_CACHE.append(build_nc())
    nc = _NC_CACHE[0]

    in_maps = [dict(in_map) for _ in range(N_CORES)]
    res = run_bass_kernel_spmd(nc, in_maps, core_ids=list(range(N_CORES)))
    return np.asarray(res.results[0]["out"], dtype=np.float32)
